# revision 24
# baseline (speedup 1.0000x reference)
"""LoRA layer kernel for Trainium2, 8-core data-parallel.

out = x @ W.T + 2.0 * ((x @ B) @ A)
  x: (4, 4096, 4096) f32, W: (4096, 4096), A: (16, 4096), B: (4096, 16)

Strategy: fold the LoRA path into the weight on the host
(W'' = W.T + 2*B@A, a 0.5-GFLOP rank-16 update), so the device runs a
single pure GEMM per core:  out_c[2048, 4096] = x_c[2048, 4096] @ W''.
Rows are sharded across the 8 cores (2048 each), W'' replicated.

Per-core kernel (bf16 inputs, fp32 PSUM accumulate), measured ~905 us
(PE streaming floor is 874 us at 2.4 GHz; 4096 matmuls at 216 ns):
  - x shard fully SBUF-resident as 64 tiles [128, 1024] bf16 (128 KiB/par)
  - loop oc(8 chunks of 512) x mg(2 row groups of 1024) x k(32):
    8 matmuls [128k,128m]x[128k,512n] accumulating into 8 PSUM banks
  - W'' oc-slice (32 tiles [128,512] bf16) loaded once per oc, reused by
    both row groups; wpool bufs=36 gives ~4 tiles of cross-oc prefetch
  - DMA ring assignment tuned so pass 0 (~229 GB/s critical traffic)
    stays under the ~358 GB/s per-core HBM cap: x mg0 half-tiles on
    gpsimd+scalar, W then x mg1 on sync (ring FIFO throttles the
    prefetch), outputs alternate scalar/sync
  - k-ascending first pass consumes x tiles as their DMAs land; matmul
    order alternates x halves to tolerate inter-ring skew
  - opool bufs=12 > 8 PSUM banks so pass-boundary drains never gate the
    next pass's start=True matmuls on out-DMA HBM-write receipts
  - final pass split [4,2,1,1] so only one PSUM drain chain (copy +
    store + receipt) remains after the last matmul
  - junk-matmul warmup trips the HAM clock gate (1.2->2.4 GHz) while
    the first x tiles are still in flight
"""

import sys

if "/opt/trn_rl_repo" not in sys.path:
    sys.path.insert(0, "/opt/trn_rl_repo")

import numpy as np

import concourse.bass as bass
import concourse.mybir as mybir
import concourse.tile as tile

N_CORES = 8
D = 4096
ROWS_TOTAL = 4 * 4096              # 16384
ROWS_PER_CORE = ROWS_TOTAL // N_CORES  # 2048
P = 128
KT = D // P                        # 32 k-tiles
MG = 2                             # row groups per core
MG_ROWS = ROWS_PER_CORE // MG      # 1024
MT = MG_ROWS // P                  # 8 m-tiles (PSUM banks) per group
OC = 512                           # o-chunk width (one PSUM bank)
N_OC = D // OC                     # 8

F32 = mybir.dt.float32
BF16 = mybir.dt.bfloat16

N_WARMUP = 24


def split_wide_waits(nc, max_waits=1):
    """walrus in this container rejects >1 sync wait per instruction;
    move excess waits onto preceding same-engine NoOps."""
    n_split = 0
    for f in nc.m.functions:
        for bb in f.blocks:
            new_insts = []
            for inst in bb.instructions:
                si = getattr(inst, "sync_info", None)
                if si is not None and si.on_wait and len(si.on_wait) > max_waits:
                    waits = list(si.on_wait)
                    keep = waits[-max_waits:]
                    extra = waits[:-max_waits]
                    for i in range(0, len(extra), max_waits):
                        chunk = extra[i:i + max_waits]
                        nop = mybir.InstNoOp(
                            name=f"{inst.name}_wsplit{i}",
                            sync_info=mybir.SyncInfo(on_wait=chunk, on_update=[]),
                            bass_nofuse=True,
                            engine=inst.engine,
                        )
                        new_insts.append(nop)
                        n_split += 1
                    si.on_wait = keep
                new_insts.append(inst)
            bb.instructions[:] = new_insts
    return n_split


def build_program():
    nc = bass.Bass()
    xt = nc.declare_dram_parameter("xt", [D, ROWS_PER_CORE], BF16, isOutput=False)
    wt = nc.declare_dram_parameter("wt", [D, D], BF16, isOutput=False)
    out = nc.declare_dram_parameter("out", [ROWS_PER_CORE, D], F32, isOutput=True)

    with tile.TileContext(nc) as tc:
        with (
            tc.tile_pool(name="xpool", bufs=KT * MG) as xpool,
            tc.tile_pool(name="wpool", bufs=36) as wpool,
            tc.tile_pool(name="opool", bufs=12) as opool,
            tc.tile_pool(name="wupool", bufs=1) as wupool,
            tc.tile_pool(name="ppool", bufs=8, space="PSUM") as ppool,
        ):
            # HAM warmup: junk matmuls trip the PE clock gate to 8/8
            # while the first x tiles stream in.
            wu = wupool.tile([P, P], BF16, tag="wu")
            nc.gpsimd.memset(wu[:], 0.0)
            junk = ppool.tile([P, P], F32, tag="acc", name="junk")
            for i in range(N_WARMUP):
                nc.tensor.matmul(
                    junk[:],
                    wu[:],
                    wu[:],
                    start=(i == 0),
                    stop=(i == N_WARMUP - 1),
                )

            # x shard resident: 64 tiles [128, 1024], issued in the order
            # the first pass consumes them (mg-major, k-ascending).
            # mg0 goes on the gpsimd/scalar rings; W oc0 is hoisted onto
            # the sync ring first, with mg1's x behind it (per-ring FIFO
            # keeps the mg1 prefetch from stealing HBM bandwidth from
            # pass 0's just-in-time W stream).
            xtiles = [[None] * KT for _ in range(MG)]
            K_TAIL = 24  # mg0 k>=24 ride the sync ring (needed late)
            for k in range(K_TAIL):
                t = xpool.tile([P, MG_ROWS], BF16, tag="x")
                # half-tile per ring: both rings deliver each k in
                # lockstep, and the first 4 m-tiles' matmuls only need
                # the first half
                half = MG_ROWS // 2
                nc.gpsimd.dma_start(
                    t[:, 0:half], xt[k * P:(k + 1) * P, 0:half]
                )
                nc.scalar.dma_start(
                    t[:, half:MG_ROWS], xt[k * P:(k + 1) * P, half:MG_ROWS]
                )
                xtiles[0][k] = t

            wtiles0 = [None] * KT
            for k in range(KT):
                w = wpool.tile([P, OC], BF16, tag="wt")
                nc.sync.dma_start(w[:], wt[k * P:(k + 1) * P, 0:OC])
                wtiles0[k] = w

            # mg0 tail k-tiles behind W oc0 on sync: shortens the
            # gpsimd/scalar critical queues 25% without starving W
            for k in range(K_TAIL, KT):
                t = xpool.tile([P, MG_ROWS], BF16, tag="x")
                nc.sync.dma_start(t[:], xt[k * P:(k + 1) * P, 0:MG_ROWS])
                xtiles[0][k] = t

            # mg1 prefetch on the sync ring behind W oc0: per-ring FIFO
            # keeps it from starving pass 0's just-in-time W stream, and
            # it still lands well before every pass-1 deadline
            for k in range(KT):
                t = xpool.tile([P, MG_ROWS], BF16, tag="x")
                nc.sync.dma_start(
                    t[:], xt[k * P:(k + 1) * P, MG_ROWS:2 * MG_ROWS]
                )
                xtiles[1][k] = t

            def half_pass(oc, mg, mts, wtiles, load_w):
                psums = [
                    ppool.tile([P, OC], F32, tag="acc",
                               name=f"ps_{oc}_{mg}_{mt}")
                    for mt in mts
                ]
                # pass 0 is paced by the two x half-tile DMA rings;
                # alternating halves doubles the per-ring deadline slack
                first = (oc == 0 and mg == 0)
                order = list(range(len(mts)))
                if first and len(mts) == MT:
                    order = [4, 0, 5, 1, 6, 2, 7, 3]
                for k in range(KT):
                    if load_w:
                        w = wpool.tile([P, OC], BF16, tag="wt")
                        nc.sync.dma_start(
                            w[:],
                            wt[k * P:(k + 1) * P, oc * OC:(oc + 1) * OC],
                        )
                        wtiles[k] = w
                    xk = xtiles[mg][k]
                    for i in order:
                        mt = mts[i]
                        nc.tensor.matmul(
                            psums[i][:],
                            xk[:, mt * P:(mt + 1) * P],
                            wtiles[k][:],
                            start=(k == 0),
                            stop=(k == KT - 1),
                        )
                for i, mt in enumerate(mts):
                    ot = opool.tile([P, OC], F32, tag="ot")
                    nc.vector.tensor_copy(ot[:], psums[i][:])
                    # alternate HWDGE rings so the drain chain at
                    # pass boundaries / kernel tail is half as deep
                    eng = nc.scalar if mt % 2 == 0 else nc.sync
                    eng.dma_start(
                        out[mg * MG_ROWS + mt * P:
                            mg * MG_ROWS + (mt + 1) * P,
                            oc * OC:(oc + 1) * OC],
                        ot[:],
                    )

            for oc in range(N_OC):
                wtiles = wtiles0 if oc == 0 else [None] * KT
                for mg in range(MG):
                    last = (oc == N_OC - 1 and mg == MG - 1)
                    load_w = (mg == 0 and oc > 0)
                    if last:
                        # split the final pass so only a sliver of the
                        # PSUM drain chain remains after the last matmul
                        half_pass(oc, mg, [0, 1, 2, 3], wtiles, load_w)
                        half_pass(oc, mg, [4, 5], wtiles, False)
                        half_pass(oc, mg, [6], wtiles, False)
                        half_pass(oc, mg, [7], wtiles, False)
                    else:
                        half_pass(oc, mg, list(range(MT)), wtiles, load_w)

    split_wide_waits(nc)
    return nc


_NC_CACHE = [None]


def kernel(x, weight, lora_A, lora_B):
    import ml_dtypes
    from concourse.bass_utils import run_bass_kernel_spmd

    bf16 = ml_dtypes.bfloat16

    x = np.asarray(x, dtype=np.float32)
    weight = np.asarray(weight, dtype=np.float32)
    lora_A = np.asarray(lora_A, dtype=np.float32)
    lora_B = np.asarray(lora_B, dtype=np.float32)

    # fold LoRA: out = x @ (W.T + 2*B@A)
    wfold = weight.T + 2.0 * (lora_B @ lora_A)
    wt = np.ascontiguousarray(wfold.astype(bf16))

    x2 = x.reshape(ROWS_TOTAL, D)
    xt_all = x2.T.astype(bf16)     # [D, ROWS_TOTAL] C-contiguous

    in_maps = []
    for c in range(N_CORES):
        xt_c = np.ascontiguousarray(
            xt_all[:, c * ROWS_PER_CORE:(c + 1) * ROWS_PER_CORE]
        )
        in_maps.append({"xt": xt_c, "wt": wt})

    if _NC_CACHE[0] is None:
        _NC_CACHE[0] = build_program()
    nc = _NC_CACHE[0]

    res = run_bass_kernel_spmd(nc, in_maps, list(range(N_CORES)))
    out = np.concatenate(
        [res.results[c]["out"] for c in range(N_CORES)], axis=0
    )
    return out.reshape(x.shape)


# revision 25
# speedup vs baseline: 1.0079x; 1.0079x over previous
"""LoRA layer kernel for Trainium2, 8-core data-parallel.

out = x @ W.T + 2.0 * ((x @ B) @ A)
  x: (4, 4096, 4096) f32, W: (4096, 4096), A: (16, 4096), B: (4096, 16)

Strategy: fold the LoRA path into the weight on the host
(W'' = W.T + 2*B@A, a 0.5-GFLOP rank-16 update), so the device runs a
single pure GEMM per core:  out_c[2048, 4096] = x_c[2048, 4096] @ W''.
Rows are sharded across the 8 cores (2048 each), W'' replicated.

Per-core kernel (bf16 inputs, fp32 PSUM accumulate), measured ~905 us
(PE streaming floor is 874 us at 2.4 GHz; 4096 matmuls at 216 ns):
  - x shard fully SBUF-resident as 64 tiles [128, 1024] bf16 (128 KiB/par)
  - loop oc(8 chunks of 512) x mg(2 row groups of 1024) x k(32):
    8 matmuls [128k,128m]x[128k,512n] accumulating into 8 PSUM banks
  - W'' oc-slice (32 tiles [128,512] bf16) loaded once per oc, reused by
    both row groups; wpool bufs=36 gives ~4 tiles of cross-oc prefetch
  - DMA ring assignment tuned so pass 0 (~229 GB/s critical traffic)
    stays under the ~358 GB/s per-core HBM cap: x mg0 half-tiles on
    gpsimd+scalar, W then x mg1 on sync (ring FIFO throttles the
    prefetch), outputs alternate scalar/sync
  - k-ascending first pass consumes x tiles as their DMAs land; matmul
    order alternates x halves to tolerate inter-ring skew
  - opool bufs=12 > 8 PSUM banks so pass-boundary drains never gate the
    next pass's start=True matmuls on out-DMA HBM-write receipts
  - final pass split [4,2,1,1] so only one PSUM drain chain (copy +
    store + receipt) remains after the last matmul
  - junk-matmul warmup trips the HAM clock gate (1.2->2.4 GHz) while
    the first x tiles are still in flight
"""

import sys

if "/opt/trn_rl_repo" not in sys.path:
    sys.path.insert(0, "/opt/trn_rl_repo")

import numpy as np

import concourse.bass as bass
import concourse.mybir as mybir
import concourse.tile as tile

N_CORES = 8
D = 4096
ROWS_TOTAL = 4 * 4096              # 16384
ROWS_PER_CORE = ROWS_TOTAL // N_CORES  # 2048
P = 128
KT = D // P                        # 32 k-tiles
MG = 2                             # row groups per core
MG_ROWS = ROWS_PER_CORE // MG      # 1024
MT = MG_ROWS // P                  # 8 m-tiles (PSUM banks) per group
OC = 512                           # o-chunk width (one PSUM bank)
N_OC = D // OC                     # 8

F32 = mybir.dt.float32
BF16 = mybir.dt.bfloat16

N_WARMUP = 24


def split_wide_waits(nc, max_waits=1):
    """walrus in this container rejects >1 sync wait per instruction;
    move excess waits onto preceding same-engine NoOps."""
    n_split = 0
    for f in nc.m.functions:
        for bb in f.blocks:
            new_insts = []
            for inst in bb.instructions:
                si = getattr(inst, "sync_info", None)
                if si is not None and si.on_wait and len(si.on_wait) > max_waits:
                    waits = list(si.on_wait)
                    keep = waits[-max_waits:]
                    extra = waits[:-max_waits]
                    for i in range(0, len(extra), max_waits):
                        chunk = extra[i:i + max_waits]
                        nop = mybir.InstNoOp(
                            name=f"{inst.name}_wsplit{i}",
                            sync_info=mybir.SyncInfo(on_wait=chunk, on_update=[]),
                            bass_nofuse=True,
                            engine=inst.engine,
                        )
                        new_insts.append(nop)
                        n_split += 1
                    si.on_wait = keep
                new_insts.append(inst)
            bb.instructions[:] = new_insts
    return n_split


def build_program():
    nc = bass.Bass()
    xt = nc.declare_dram_parameter("xt", [D, ROWS_PER_CORE], BF16, isOutput=False)
    wt = nc.declare_dram_parameter("wt", [D, D], BF16, isOutput=False)
    out = nc.declare_dram_parameter("out", [ROWS_PER_CORE, D], F32, isOutput=True)

    with tile.TileContext(nc) as tc:
        with (
            tc.tile_pool(name="xpool", bufs=KT * MG) as xpool,
            tc.tile_pool(name="wpool", bufs=36) as wpool,
            tc.tile_pool(name="opool", bufs=12) as opool,
            tc.tile_pool(name="wupool", bufs=1) as wupool,
            tc.tile_pool(name="ppool", bufs=8, space="PSUM") as ppool,
        ):
            # HAM warmup: junk matmuls trip the PE clock gate to 8/8
            # while the first x tiles stream in.
            wu = wupool.tile([P, P], BF16, tag="wu")
            nc.gpsimd.memset(wu[:], 0.0)
            junk = ppool.tile([P, P], F32, tag="acc", name="junk")
            for i in range(N_WARMUP):
                nc.tensor.matmul(
                    junk[:],
                    wu[:],
                    wu[:],
                    start=(i == 0),
                    stop=(i == N_WARMUP - 1),
                )

            # x shard resident: 64 tiles [128, 1024], issued in the order
            # the first pass consumes them (mg-major, k-ascending).
            # mg0 goes on the gpsimd/scalar rings; W oc0 is hoisted onto
            # the sync ring first, with mg1's x behind it (per-ring FIFO
            # keeps the mg1 prefetch from stealing HBM bandwidth from
            # pass 0's just-in-time W stream).
            xtiles = [[None] * KT for _ in range(MG)]
            for k in range(KT):
                t = xpool.tile([P, MG_ROWS], BF16, tag="x")
                # half-tile per ring: both rings deliver each k in
                # lockstep, and the first 4 m-tiles' matmuls only need
                # the first half
                half = MG_ROWS // 2
                nc.gpsimd.dma_start(
                    t[:, 0:half], xt[k * P:(k + 1) * P, 0:half]
                )
                nc.scalar.dma_start(
                    t[:, half:MG_ROWS], xt[k * P:(k + 1) * P, half:MG_ROWS]
                )
                xtiles[0][k] = t

            wtiles0 = [None] * KT
            for k in range(KT):
                w = wpool.tile([P, OC], BF16, tag="wt")
                nc.sync.dma_start(w[:], wt[k * P:(k + 1) * P, 0:OC])
                wtiles0[k] = w

            # mg1 prefetch on the sync ring behind W oc0: per-ring FIFO
            # keeps it from starving pass 0's just-in-time W stream, and
            # it still lands well before every pass-1 deadline
            for k in range(KT):
                t = xpool.tile([P, MG_ROWS], BF16, tag="x")
                nc.sync.dma_start(
                    t[:], xt[k * P:(k + 1) * P, MG_ROWS:2 * MG_ROWS]
                )
                xtiles[1][k] = t

            def half_pass(oc, mg, mts, wtiles, load_w):
                psums = [
                    ppool.tile([P, OC], F32, tag="acc",
                               name=f"ps_{oc}_{mg}_{mt}")
                    for mt in mts
                ]
                # pass 0 is paced by the two x half-tile DMA rings;
                # alternating halves doubles the per-ring deadline slack
                first = (oc == 0 and mg == 0)
                order = list(range(len(mts)))
                if first and len(mts) == MT:
                    order = [4, 0, 5, 1, 6, 2, 7, 3]
                for k in range(KT):
                    if load_w:
                        w = wpool.tile([P, OC], BF16, tag="wt")
                        nc.sync.dma_start(
                            w[:],
                            wt[k * P:(k + 1) * P, oc * OC:(oc + 1) * OC],
                        )
                        wtiles[k] = w
                    xk = xtiles[mg][k]
                    for i in order:
                        mt = mts[i]
                        nc.tensor.matmul(
                            psums[i][:],
                            xk[:, mt * P:(mt + 1) * P],
                            wtiles[k][:],
                            start=(k == 0),
                            stop=(k == KT - 1),
                        )
                for i, mt in enumerate(mts):
                    ot = opool.tile([P, OC], F32, tag="ot")
                    nc.vector.tensor_copy(ot[:], psums[i][:])
                    # alternate HWDGE rings so the drain chain at
                    # pass boundaries / kernel tail is half as deep
                    eng = nc.scalar if mt % 2 == 0 else nc.sync
                    eng.dma_start(
                        out[mg * MG_ROWS + mt * P:
                            mg * MG_ROWS + (mt + 1) * P,
                            oc * OC:(oc + 1) * OC],
                        ot[:],
                    )

            for oc in range(N_OC):
                wtiles = wtiles0 if oc == 0 else [None] * KT
                for mg in range(MG):
                    last = (oc == N_OC - 1 and mg == MG - 1)
                    load_w = (mg == 0 and oc > 0)
                    if last:
                        # split the final pass so only a sliver of the
                        # PSUM drain chain remains after the last matmul
                        half_pass(oc, mg, [0, 1, 2, 3], wtiles, load_w)
                        half_pass(oc, mg, [4, 5], wtiles, False)
                        half_pass(oc, mg, [6], wtiles, False)
                        half_pass(oc, mg, [7], wtiles, False)
                    else:
                        half_pass(oc, mg, list(range(MT)), wtiles, load_w)

    split_wide_waits(nc)
    return nc


_NC_CACHE = [None]


def kernel(x, weight, lora_A, lora_B):
    import ml_dtypes
    from concourse.bass_utils import run_bass_kernel_spmd

    bf16 = ml_dtypes.bfloat16

    x = np.asarray(x, dtype=np.float32)
    weight = np.asarray(weight, dtype=np.float32)
    lora_A = np.asarray(lora_A, dtype=np.float32)
    lora_B = np.asarray(lora_B, dtype=np.float32)

    # fold LoRA: out = x @ (W.T + 2*B@A)
    wfold = weight.T + 2.0 * (lora_B @ lora_A)
    wt = np.ascontiguousarray(wfold.astype(bf16))

    x2 = x.reshape(ROWS_TOTAL, D)
    xt_all = x2.T.astype(bf16)     # [D, ROWS_TOTAL] C-contiguous

    in_maps = []
    for c in range(N_CORES):
        xt_c = np.ascontiguousarray(
            xt_all[:, c * ROWS_PER_CORE:(c + 1) * ROWS_PER_CORE]
        )
        in_maps.append({"xt": xt_c, "wt": wt})

    if _NC_CACHE[0] is None:
        _NC_CACHE[0] = build_program()
    nc = _NC_CACHE[0]

    res = run_bass_kernel_spmd(nc, in_maps, list(range(N_CORES)))
    out = np.concatenate(
        [res.results[c]["out"] for c in range(N_CORES)], axis=0
    )
    return out.reshape(x.shape)
